# revision 5
# baseline (speedup 1.0000x reference)
"""SSIM loss kernel for Trainium2 (Bass/Tile), 8-core data parallel.

Math (scale-free "starred" units; C1=1e-4, C2=9e-4, CA=2*C1, CB=2*C2):
  F1 = X+Y, F2 = X-Y, uF = F1^2, vF = F2^2
  pass1 (row blur, transposing banded matmuls):  Ts=blur(F1), Td=blur(F2),
      TP=blur(uF)+blur(vF), TM=blur(uF)-blur(vF)   (TP/TM via +/- band 2-src)
  pass2 (col blur, transposing):  psS, psD, psP, psM
  u = psS^2, v = psD^2 ; a1 = u-v (= 4 mu_x mu_y), b1 = u+v
  Nt = (a1+CA)(psM*g2 - a1 + CB) = 4*A1*A2     (one fused custom-DVE op)
  Dt = (b1+CA)(psP*g2 - b1 + CB) = 4*B1*B2     (same op)
  loss = 1 - mean(Nt/Dt)
g2 corrects the bf16 band's row-sum gain: sigma terms mix once-blurred (gain G)
with squared-blurred (gain G^2) quantities; scaling psP/psM by G re-matches
them (without it the bias is ~4.6e-3 on the loss; with it ~3e-4).

All matmuls bf16 (1 cyc/col vs fp32's 4), banded single-window-per-k-tile
("b4": 542 streamed cols per conv vs 2048 dense).  The divide+multiply+
reduce runs as ONE custom-DVE op (BITWISE_NOT-seeded reciprocal, 1 inline NR
pass, ~1.7e-3 max rel err — noise for a 4M-pixel mean).  Engine split:
PE convs; DVE F1/F2/uF prep, (b1|a1), (Dt|Nt), div+reduce; Act vF, pass-1
drains, (u|v) squares; SP DMA.  Measured ~122 us/core (baseline fp32: 477).
"""

import sys

for _p in ("/opt/trn_rl_repo",):
    if _p not in sys.path:
        sys.path.insert(0, _p)

import numpy as np
import ml_dtypes

import concourse.bass as bass
import concourse.bacc as bacc
import concourse.mybir as mybir
import concourse.tile as tile
from concourse.bass_utils import run_bass_kernel_spmd

F32 = mybir.dt.float32
BF16 = mybir.dt.bfloat16
AOP = mybir.AluOpType
AFT = mybir.ActivationFunctionType

N_CORES = 8
BATCH = 16
CH = 3
H = W = 512
PLANES = (BATCH // N_CORES) * CH  # 6 planes per core
WIN_SIZE = 11
SIGMA = 1.5
HALF = WIN_SIZE // 2
C1 = 0.01 ** 2
C2 = 0.03 ** 2
CA = 2.0 * C1
CB = 2.0 * C2

# banded window per k-tile: output rows [ns, ns+w) reachable from that k-tile
BWIN = [(0, 133), (123, 138), (251, 138), (379, 133)]
BOFF = [0, 133, 271, 409]
CATW = 542

# 1-pass Chebyshev pair for the BITWISE_NOT reciprocal seed
DVA_C0 = -0.23549792
DVA_C1 = 2.0017324


def _gauss1d():
    coords = np.arange(WIN_SIZE, dtype=np.float32) - HALF
    g = np.exp(-(coords ** 2) / np.float32(2.0 * SIGMA ** 2)).astype(np.float32)
    return (g / g.sum(dtype=np.float32)).astype(np.float32)


def _band_full():
    g = _gauss1d()
    A = np.zeros((H, H), dtype=np.float32)
    for i in range(H):
        lo, hi = max(0, i - HALF), min(H, i + HALF + 1)
        for j in range(lo, hi):
            A[i, j] = g[j - i + HALF]
    return A


def band_banded_bf16():
    """[128, 2*542] bf16: banded concat segments then negated copy."""
    A = _band_full()
    segs = []
    for kt in range(4):
        ns, w = BWIN[kt]
        segs.append(A[ns:ns + w, kt * 128:(kt + 1) * 128].T.copy())
    cat = np.concatenate(segs, axis=1)
    assert cat.shape == (128, CATW)
    return np.concatenate([cat, -cat], axis=1).astype(ml_dtypes.bfloat16)


def _gain2():
    """Interior blur2 gain of the bf16 band (row-sum squared)."""
    g = _gauss1d().astype(ml_dtypes.bfloat16).astype(np.float64)
    s1 = float(g.sum())
    return s1 * s1


_NT_OP = None
_DVA_OP = None


def _register_nt_op():
    """out = (in1+s0)*(in0*imm2 - in1 + s1) — the fused SSIM numerator/
    denominator term. Registered at the first free custom-DVE opcode row."""
    global _NT_OP
    if _NT_OP is not None:
        return _NT_OP
    from concourse.dve_ops import DveOp, OPS, _SUB_OPCODE_FOR_NAME, has_src1
    from concourse.dve_spec import Spec, Src0, Src1, C0, C1 as SC1, C2 as SC2, lower
    from concourse.dve_uop import DveOpSpec

    name = "NT_SSIM_ANT"
    if name in _SUB_OPCODE_FOR_NAME:
        _NT_OP = next(o for o in OPS if o.name == name)
        return _NT_OP
    row = max(_SUB_OPCODE_FOR_NAME.values()) + 1
    assert row < 0x20, "no free custom-DVE opcode row"

    def _ref(in0, in1, s0, s1, imm2):
        a = in1.astype(np.float32)
        return (a + s0) * (in0.astype(np.float32) * imm2 - a + s1)

    spec = Spec(body=(Src1 + C0) * (Src0 * SC2 - Src1 + SC1), reference=_ref)
    shas = {}
    for ver in ("v3",):
        s = DveOpSpec(name=name, opcode=row,
                      uops=lower(spec, ver=ver), rd1_en=has_src1(spec))
        shas[ver] = s.sha(ver)
    _SUB_OPCODE_FOR_NAME[name] = row
    op = DveOp(name, spec, subdim=False, uops_sha=shas)
    OPS.append(op)
    _NT_OP = op
    return op


def _register_dva_op():
    """out = Src1 * recip1(Src0); accum_out = sum(out).  recip1 = the
    BITWISE_NOT-seeded approximate reciprocal with one inline NR pass."""
    global _DVA_OP
    if _DVA_OP is not None:
        return _DVA_OP
    from concourse.dve_ops import DveOp, OPS, _SUB_OPCODE_FOR_NAME, has_src1
    from concourse.dve_spec import (
        Spec, Src0, Src1, C0, C1 as SC1, lower, Bin, AluOp,
    )
    from concourse.dve_uop import DveOpSpec
    import operator

    name = "DIV_MUL_ACC_ANT"
    if name in _SUB_OPCODE_FOR_NAME:
        _DVA_OP = next(o for o in OPS if o.name == name)
        return _DVA_OP
    row = max(_SUB_OPCODE_FOR_NAME.values()) + 1
    assert row < 0x20

    _nx = Bin(AluOp.BITWISE_NOT, Src0, Src0)
    _z0 = _nx * C0
    _z1 = _z0 * (SC1 - Src0 * _z0)
    spec = Spec(body=Src1 * _z1, accum=operator.add)
    shas = {}
    for ver in ("v3",):
        s = DveOpSpec(name=name, opcode=row,
                      uops=lower(spec, ver=ver), rd1_en=has_src1(spec))
        shas[ver] = s.sha(ver)
    _SUB_OPCODE_FOR_NAME[name] = row
    op = DveOp(name, spec, subdim=False, uops_sha=shas)
    OPS.append(op)
    _DVA_OP = op
    return op


def build_nc(planes=PLANES):
    nt_op = _register_nt_op()
    dva_op = _register_dva_op()
    g2 = _gain2()

    nc = bacc.Bacc(None)
    dmae = nc.sync
    pred_d = nc.declare_dram_parameter("pred", [planes, H, W], F32, isOutput=False)
    targ_d = nc.declare_dram_parameter("target", [planes, H, W], F32, isOutput=False)
    band_d = nc.declare_dram_parameter("bandmat", [128, 2 * CATW], BF16,
                                       isOutput=False)
    acc_d = nc.declare_dram_parameter("acc", [128, 2 * planes], F32, isOutput=True)

    with tile.TileContext(nc) as tc:
        with (
            tc.tile_pool(name="const", bufs=1) as constp,
            tc.tile_pool(name="xy", bufs=2) as xyp,
            tc.tile_pool(name="fields", bufs=3) as fldp,
            tc.tile_pool(name="tt", bufs=3) as ttp,
            tc.tile_pool(name="post", bufs=2) as pp,
            tc.tile_pool(name="postf", bufs=1) as ppf,
            tc.tile_pool(name="accp", bufs=1) as accp,
            tc.tile_pool(name="ps1", bufs=2, space="PSUM") as ps1,
            tc.tile_pool(name="ps2", bufs=2, space="PSUM") as ps2,
        ):
            BM = constp.tile([128, 2 * CATW], BF16)
            # Act HWDGE queue: keeps the sync queue free for plane-0 X/Y
            nc.scalar.dma_start(BM[:], band_d[:])
            acc = accp.tile([128, 2 * planes], F32)

            def conv_pass(dst_psum, srcs, blk):
                """dst_psum[128, 512] += transposing 1-D conv along partition
                axis of each (tile, negated) source's 128-col block `blk`."""
                mms = []
                for si, (T, neg) in enumerate(srcs):
                    for kt in range(4):
                        lhsT = T[:, kt * 512 + blk * 128: kt * 512 + (blk + 1) * 128]
                        ns, w = BWIN[kt]
                        off = BOFF[kt] + (CATW if neg else 0)
                        mms.append((dst_psum[:, ns:ns + w],
                                    lhsT, BM[:, off:off + w]))
                n_mm = len(mms)
                for i, (o, l, r) in enumerate(mms):
                    nc.tensor.matmul(o, l, r, start=(i == 0), stop=(i == n_mm - 1))

            for p in range(planes):
                X = xyp.tile([128, 2048], F32, tag="X")
                Y = xyp.tile([128, 2048], F32, tag="Y")
                dmae.dma_start(
                    X[:].rearrange("q (kt c) -> q kt c", kt=4),
                    pred_d[p].rearrange("(kt q) c -> q kt c", q=128))
                # plane 0: Act HWDGE queue runs Y in parallel with X on the
                # sync queue, shortening the pipeline-fill head
                ydma = nc.scalar if p == 0 else dmae
                ydma.dma_start(
                    Y[:].rearrange("q (kt c) -> q kt c", kt=4),
                    targ_d[p].rearrange("(kt q) c -> q kt c", q=128))

                F1 = fldp.tile([128, 2048], BF16, tag="F1")
                F2 = fldp.tile([128, 2048], BF16, tag="F2")
                uF = fldp.tile([128, 2048], BF16, tag="uF")
                vF = fldp.tile([128, 2048], BF16, tag="vF")
                nc.vector.tensor_tensor(F1[:], X[:], Y[:], AOP.add)
                nc.vector.tensor_tensor(F2[:], X[:], Y[:], AOP.subtract)
                nc.vector.tensor_tensor(uF[:], F1[:], F1[:], AOP.mult)
                nc.scalar.activation(vF[:], F2[:], AFT.Square)

                # pass 1: row blur (transposing), [128,1024] PSUM grain
                p1_specs = [
                    ("Ts", [(F1, False)]),
                    ("Td", [(F2, False)]),
                    ("TP", [(uF, False), (vF, False)]),
                    ("TM", [(uF, False), (vF, True)]),
                ]
                Ts = {}
                for nm, srcs in p1_specs:
                    T = ttp.tile([128, 2048], BF16, tag=nm)
                    for half in range(2):
                        ps = ps1.tile([128, 1024], F32, tag="p1")
                        for i in range(2):
                            mc = 2 * half + i
                            conv_pass(ps[:, i * 512:(i + 1) * 512], srcs, mc)
                        nc.scalar.copy(T[:, half * 1024:(half + 1) * 1024], ps[:])
                    Ts[nm] = T

                # pass 2: col blur + fused post-processing
                uv = pp.tile([128, 4096], BF16, tag="uv")
                ba = pp.tile([128, 4096], BF16, tag="ba")
                ND = ppf.tile([128, 4096], F32, tag="ND")
                scr = ppf.tile([128, 2048], BF16, tag="scr")
                for rc in range(4):
                    o = rc * 1024
                    sl_u = slice(o, o + 512)
                    sl_v = slice(o + 512, o + 1024)
                    psSD = ps2.tile([128, 1024], F32, tag="p2")
                    conv_pass(psSD[:, 0:512], [(Ts["Ts"], False)], rc)
                    conv_pass(psSD[:, 512:1024], [(Ts["Td"], False)], rc)
                    nc.scalar.activation(uv[:, o:o + 1024], psSD[:], AFT.Square)
                    nc.vector.tensor_tensor(
                        ba[:, sl_u], uv[:, sl_u], uv[:, sl_v], AOP.add)
                    nc.vector.tensor_tensor(
                        ba[:, sl_v], uv[:, sl_u], uv[:, sl_v], AOP.subtract)
                    psPM = ps2.tile([128, 1024], F32, tag="p2")
                    conv_pass(psPM[:, 0:512], [(Ts["TP"], False)], rc)
                    conv_pass(psPM[:, 512:1024], [(Ts["TM"], False)], rc)
                    nc.vector._custom_dve(
                        nt_op, out=ND[:, o:o + 1024], in0=psPM[:],
                        in1=ba[:, o:o + 1024], s0=CA, s1=CB, imm2=g2)

                for hf in range(2):
                    hs = slice(hf * 2048, hf * 2048 + 2048)
                    ND_r = ND[:, hs].rearrange("q (r n) -> q r n", r=2)
                    nc.vector._custom_dve(
                        dva_op,
                        out=scr[:, hf * 1024:(hf + 1) * 1024].rearrange(
                            "q (r n) -> q r n", r=2),
                        in0=ND_r[:, :, 0:512], in1=ND_r[:, :, 512:1024],
                        s0=DVA_C0, s1=DVA_C1,
                        accum_out=acc[:, 2 * p + hf:2 * p + hf + 1])

            dmae.dma_start(acc_d[:], acc[:])
    nc.compile()
    return nc


_CACHE = {}


def _get_nc():
    if "nc" not in _CACHE:
        _CACHE["nc"] = build_nc()
        _CACHE["band"] = band_banded_bf16()
    return _CACHE["nc"], _CACHE["band"]


def kernel(pred, target, _trace=False):
    pred = np.ascontiguousarray(np.asarray(pred), dtype=np.float32)
    target = np.ascontiguousarray(np.asarray(target), dtype=np.float32)
    nc, band = _get_nc()
    per = BATCH // N_CORES
    in_maps = []
    for i in range(N_CORES):
        in_maps.append({
            "pred": np.ascontiguousarray(
                pred[per * i: per * (i + 1)].reshape(PLANES, H, W)),
            "target": np.ascontiguousarray(
                target[per * i: per * (i + 1)].reshape(PLANES, H, W)),
            "bandmat": band,
        })
    kw = {}
    if _trace:
        kw["trace"] = True
    res = run_bass_kernel_spmd(nc, in_maps, list(range(N_CORES)), **kw)
    total = 0.0
    for r in res.results:
        total += float(np.asarray(r["acc"]).astype(np.float64).sum())
    loss = 1.0 - total / float(BATCH * CH * H * W)
    out = np.float32(loss)
    if _trace:
        return out, res
    return out


# revision 6
# speedup vs baseline: 1.0173x; 1.0173x over previous
"""SSIM loss kernel for Trainium2 (Bass/Tile), 8-core data parallel.

Math (scale-free "starred" units; C1=1e-4, C2=9e-4, CA=2*C1, CB=2*C2):
  F1 = X+Y, F2 = X-Y, uF = F1^2, vF = F2^2
  pass1 (row blur, transposing banded matmuls):  Ts=blur(F1), Td=blur(F2),
      TP=blur(uF)+blur(vF), TM=blur(uF)-blur(vF)   (TP/TM via +/- band 2-src)
  pass2 (col blur, transposing):  psS, psD, psP, psM
  u = psS^2, v = psD^2 ; a1 = u-v (= 4 mu_x mu_y), b1 = u+v
  Nt = (a1+CA)(psM*g2 - a1 + CB) = 4*A1*A2     (one fused custom-DVE op)
  Dt = (b1+CA)(psP*g2 - b1 + CB) = 4*B1*B2     (same op)
  loss = 1 - mean(Nt/Dt)
g2 corrects the bf16 band's row-sum gain: sigma terms mix once-blurred (gain G)
with squared-blurred (gain G^2) quantities; scaling psP/psM by G re-matches
them (without it the bias is ~4.6e-3 on the loss; with it ~3e-4).

All matmuls bf16 (1 cyc/col vs fp32's 4), banded single-window-per-k-tile
("b4": 542 streamed cols per conv vs 2048 dense).  The divide+multiply+
reduce runs as ONE custom-DVE op (BITWISE_NOT-seeded reciprocal, 1 inline NR
pass, ~1.7e-3 max rel err — noise for a 4M-pixel mean).  Engine split:
PE convs; DVE F1/F2/uF prep, (b1|a1), (Dt|Nt), div+reduce; Act vF, pass-1
drains, (u|v) squares; SP DMA.  Measured ~122 us/core (baseline fp32: 477).
"""

import sys

for _p in ("/opt/trn_rl_repo",):
    if _p not in sys.path:
        sys.path.insert(0, _p)

import numpy as np
import ml_dtypes

import concourse.bass as bass
import concourse.bacc as bacc
import concourse.mybir as mybir
import concourse.tile as tile
from concourse.bass_utils import run_bass_kernel_spmd

F32 = mybir.dt.float32
BF16 = mybir.dt.bfloat16
AOP = mybir.AluOpType
AFT = mybir.ActivationFunctionType

N_CORES = 8
BATCH = 16
CH = 3
H = W = 512
PLANES = (BATCH // N_CORES) * CH  # 6 planes per core
WIN_SIZE = 11
SIGMA = 1.5
HALF = WIN_SIZE // 2
C1 = 0.01 ** 2
C2 = 0.03 ** 2
CA = 2.0 * C1
CB = 2.0 * C2

# banded window per k-tile: output rows [ns, ns+w) reachable from that k-tile
BWIN = [(0, 133), (123, 138), (251, 138), (379, 133)]
BOFF = [0, 133, 271, 409]
CATW = 542

# 1-pass Chebyshev pair for the BITWISE_NOT reciprocal seed
DVA_C0 = -0.23549792
DVA_C1 = 2.0017324


def _gauss1d():
    coords = np.arange(WIN_SIZE, dtype=np.float32) - HALF
    g = np.exp(-(coords ** 2) / np.float32(2.0 * SIGMA ** 2)).astype(np.float32)
    return (g / g.sum(dtype=np.float32)).astype(np.float32)


def _band_full():
    g = _gauss1d()
    A = np.zeros((H, H), dtype=np.float32)
    for i in range(H):
        lo, hi = max(0, i - HALF), min(H, i + HALF + 1)
        for j in range(lo, hi):
            A[i, j] = g[j - i + HALF]
    return A


def band_banded_bf16():
    """[128, 2*542] bf16: banded concat segments then negated copy."""
    A = _band_full()
    segs = []
    for kt in range(4):
        ns, w = BWIN[kt]
        segs.append(A[ns:ns + w, kt * 128:(kt + 1) * 128].T.copy())
    cat = np.concatenate(segs, axis=1)
    assert cat.shape == (128, CATW)
    return np.concatenate([cat, -cat], axis=1).astype(ml_dtypes.bfloat16)


def _gain2():
    """Interior blur2 gain of the bf16 band (row-sum squared)."""
    g = _gauss1d().astype(ml_dtypes.bfloat16).astype(np.float64)
    s1 = float(g.sum())
    return s1 * s1


_NT_OP = None
_DVA_OP = None


def _register_nt_op():
    """out = (in1+s0)*(in0*imm2 - in1 + s1) — the fused SSIM numerator/
    denominator term. Registered at the first free custom-DVE opcode row."""
    global _NT_OP
    if _NT_OP is not None:
        return _NT_OP
    from concourse.dve_ops import DveOp, OPS, _SUB_OPCODE_FOR_NAME, has_src1
    from concourse.dve_spec import Spec, Src0, Src1, C0, C1 as SC1, C2 as SC2, lower
    from concourse.dve_uop import DveOpSpec

    name = "NT_SSIM_ANT"
    if name in _SUB_OPCODE_FOR_NAME:
        _NT_OP = next(o for o in OPS if o.name == name)
        return _NT_OP
    row = max(_SUB_OPCODE_FOR_NAME.values()) + 1
    assert row < 0x20, "no free custom-DVE opcode row"

    def _ref(in0, in1, s0, s1, imm2):
        a = in1.astype(np.float32)
        return (a + s0) * (in0.astype(np.float32) * imm2 - a + s1)

    spec = Spec(body=(Src1 + C0) * (Src0 * SC2 - Src1 + SC1), reference=_ref)
    shas = {}
    for ver in ("v3",):
        s = DveOpSpec(name=name, opcode=row,
                      uops=lower(spec, ver=ver), rd1_en=has_src1(spec))
        shas[ver] = s.sha(ver)
    _SUB_OPCODE_FOR_NAME[name] = row
    op = DveOp(name, spec, subdim=False, uops_sha=shas)
    OPS.append(op)
    _NT_OP = op
    return op


def _register_dva_op():
    """out = Src1 * recip1(Src0); accum_out = sum(out).  recip1 = the
    BITWISE_NOT-seeded approximate reciprocal with one inline NR pass."""
    global _DVA_OP
    if _DVA_OP is not None:
        return _DVA_OP
    from concourse.dve_ops import DveOp, OPS, _SUB_OPCODE_FOR_NAME, has_src1
    from concourse.dve_spec import (
        Spec, Src0, Src1, C0, C1 as SC1, lower, Bin, AluOp,
    )
    from concourse.dve_uop import DveOpSpec
    import operator

    name = "DIV_MUL_ACC_ANT"
    if name in _SUB_OPCODE_FOR_NAME:
        _DVA_OP = next(o for o in OPS if o.name == name)
        return _DVA_OP
    row = max(_SUB_OPCODE_FOR_NAME.values()) + 1
    assert row < 0x20

    _nx = Bin(AluOp.BITWISE_NOT, Src0, Src0)
    _z0 = _nx * C0
    _z1 = _z0 * (SC1 - Src0 * _z0)
    spec = Spec(body=Src1 * _z1, accum=operator.add)
    shas = {}
    for ver in ("v3",):
        s = DveOpSpec(name=name, opcode=row,
                      uops=lower(spec, ver=ver), rd1_en=has_src1(spec))
        shas[ver] = s.sha(ver)
    _SUB_OPCODE_FOR_NAME[name] = row
    op = DveOp(name, spec, subdim=False, uops_sha=shas)
    OPS.append(op)
    _DVA_OP = op
    return op


def build_nc(planes=PLANES):
    nt_op = _register_nt_op()
    dva_op = _register_dva_op()
    g2 = _gain2()

    nc = bacc.Bacc(None)
    dmae = nc.sync
    pred_d = nc.declare_dram_parameter("pred", [planes, H, W], F32, isOutput=False)
    targ_d = nc.declare_dram_parameter("target", [planes, H, W], F32, isOutput=False)
    band_d = nc.declare_dram_parameter("bandmat", [128, 2 * CATW], BF16,
                                       isOutput=False)
    acc_d = nc.declare_dram_parameter("acc", [128, 2 * planes], F32, isOutput=True)

    with tile.TileContext(nc) as tc:
        with (
            tc.tile_pool(name="const", bufs=1) as constp,
            tc.tile_pool(name="xy", bufs=2) as xyp,
            tc.tile_pool(name="fields", bufs=3) as fldp,
            tc.tile_pool(name="tt", bufs=3) as ttp,
            tc.tile_pool(name="post", bufs=2) as pp,
            tc.tile_pool(name="postf", bufs=1) as ppf,
            tc.tile_pool(name="accp", bufs=1) as accp,
            tc.tile_pool(name="ps1", bufs=2, space="PSUM") as ps1,
            tc.tile_pool(name="ps2", bufs=2, space="PSUM") as ps2,
        ):
            BM = constp.tile([128, 2 * CATW], BF16)
            # Act HWDGE queue: keeps the sync queue free for plane-0 X/Y
            nc.scalar.dma_start(BM[:], band_d[:])
            acc = accp.tile([128, 2 * planes], F32)

            def conv_pass(dst_psum, srcs, blk):
                """dst_psum[128, 512] += transposing 1-D conv along partition
                axis of each (tile, negated) source's 128-col block `blk`."""
                mms = []
                for si, (T, neg) in enumerate(srcs):
                    for kt in range(4):
                        lhsT = T[:, kt * 512 + blk * 128: kt * 512 + (blk + 1) * 128]
                        ns, w = BWIN[kt]
                        off = BOFF[kt] + (CATW if neg else 0)
                        mms.append((dst_psum[:, ns:ns + w],
                                    lhsT, BM[:, off:off + w]))
                n_mm = len(mms)
                for i, (o, l, r) in enumerate(mms):
                    nc.tensor.matmul(o, l, r, start=(i == 0), stop=(i == n_mm - 1))

            for p in range(planes):
                X = xyp.tile([128, 2048], F32, tag="X")
                Y = xyp.tile([128, 2048], F32, tag="Y")
                dmae.dma_start(
                    X[:].rearrange("q (kt c) -> q kt c", kt=4),
                    pred_d[p].rearrange("(kt q) c -> q kt c", q=128))
                dmae.dma_start(
                    Y[:].rearrange("q (kt c) -> q kt c", kt=4),
                    targ_d[p].rearrange("(kt q) c -> q kt c", q=128))

                F1 = fldp.tile([128, 2048], BF16, tag="F1")
                F2 = fldp.tile([128, 2048], BF16, tag="F2")
                uF = fldp.tile([128, 2048], BF16, tag="uF")
                vF = fldp.tile([128, 2048], BF16, tag="vF")
                nc.vector.tensor_tensor(F1[:], X[:], Y[:], AOP.add)
                nc.vector.tensor_tensor(F2[:], X[:], Y[:], AOP.subtract)
                nc.vector.tensor_tensor(uF[:], F1[:], F1[:], AOP.mult)
                nc.scalar.activation(vF[:], F2[:], AFT.Square)

                # pass 1: row blur (transposing), [128,1024] PSUM grain
                p1_specs = [
                    ("Ts", [(F1, False)]),
                    ("Td", [(F2, False)]),
                    ("TP", [(uF, False), (vF, False)]),
                    ("TM", [(uF, False), (vF, True)]),
                ]
                Ts = {}
                for nm, srcs in p1_specs:
                    T = ttp.tile([128, 2048], BF16, tag=nm)
                    for half in range(2):
                        ps = ps1.tile([128, 1024], F32, tag="p1")
                        for i in range(2):
                            mc = 2 * half + i
                            conv_pass(ps[:, i * 512:(i + 1) * 512], srcs, mc)
                        nc.scalar.copy(T[:, half * 1024:(half + 1) * 1024], ps[:])
                    Ts[nm] = T

                # pass 2: col blur + fused post-processing
                uv = pp.tile([128, 4096], BF16, tag="uv")
                ba = pp.tile([128, 4096], BF16, tag="ba")
                ND = ppf.tile([128, 4096], F32, tag="ND")
                scr = ppf.tile([128, 2048], BF16, tag="scr")
                for rc in range(4):
                    o = rc * 1024
                    sl_u = slice(o, o + 512)
                    sl_v = slice(o + 512, o + 1024)
                    psSD = ps2.tile([128, 1024], F32, tag="p2")
                    conv_pass(psSD[:, 0:512], [(Ts["Ts"], False)], rc)
                    conv_pass(psSD[:, 512:1024], [(Ts["Td"], False)], rc)
                    nc.scalar.activation(uv[:, o:o + 1024], psSD[:], AFT.Square)
                    nc.vector.tensor_tensor(
                        ba[:, sl_u], uv[:, sl_u], uv[:, sl_v], AOP.add)
                    nc.vector.tensor_tensor(
                        ba[:, sl_v], uv[:, sl_u], uv[:, sl_v], AOP.subtract)
                    psPM = ps2.tile([128, 1024], F32, tag="p2")
                    conv_pass(psPM[:, 0:512], [(Ts["TP"], False)], rc)
                    conv_pass(psPM[:, 512:1024], [(Ts["TM"], False)], rc)
                    nc.vector._custom_dve(
                        nt_op, out=ND[:, o:o + 1024], in0=psPM[:],
                        in1=ba[:, o:o + 1024], s0=CA, s1=CB, imm2=g2)

                for hf in range(2):
                    hs = slice(hf * 2048, hf * 2048 + 2048)
                    ND_r = ND[:, hs].rearrange("q (r n) -> q r n", r=2)
                    nc.vector._custom_dve(
                        dva_op,
                        out=scr[:, hf * 1024:(hf + 1) * 1024].rearrange(
                            "q (r n) -> q r n", r=2),
                        in0=ND_r[:, :, 0:512], in1=ND_r[:, :, 512:1024],
                        s0=DVA_C0, s1=DVA_C1,
                        accum_out=acc[:, 2 * p + hf:2 * p + hf + 1])

            dmae.dma_start(acc_d[:], acc[:])
    nc.compile()
    return nc


_CACHE = {}


def _get_nc():
    if "nc" not in _CACHE:
        _CACHE["nc"] = build_nc()
        _CACHE["band"] = band_banded_bf16()
    return _CACHE["nc"], _CACHE["band"]


def kernel(pred, target, _trace=False):
    pred = np.ascontiguousarray(np.asarray(pred), dtype=np.float32)
    target = np.ascontiguousarray(np.asarray(target), dtype=np.float32)
    nc, band = _get_nc()
    per = BATCH // N_CORES
    in_maps = []
    for i in range(N_CORES):
        in_maps.append({
            "pred": np.ascontiguousarray(
                pred[per * i: per * (i + 1)].reshape(PLANES, H, W)),
            "target": np.ascontiguousarray(
                target[per * i: per * (i + 1)].reshape(PLANES, H, W)),
            "bandmat": band,
        })
    kw = {}
    if _trace:
        kw["trace"] = True
    res = run_bass_kernel_spmd(nc, in_maps, list(range(N_CORES)), **kw)
    total = 0.0
    for r in res.results:
        total += float(np.asarray(r["acc"]).astype(np.float64).sum())
    loss = 1.0 - total / float(BATCH * CH * H * W)
    out = np.float32(loss)
    if _trace:
        return out, res
    return out


# revision 8
# speedup vs baseline: 1.0188x; 1.0015x over previous
"""SSIM loss kernel for Trainium2 (Bass/Tile), 8-core data parallel.

Math (scale-free "starred" units; C1=1e-4, C2=9e-4, CA=2*C1, CB=2*C2):
  F1 = X+Y, F2 = X-Y, uF = F1^2, vF = F2^2
  pass1 (row blur, transposing banded matmuls):  Ts=blur(F1), Td=blur(F2),
      TP=blur(uF)+blur(vF), TM=blur(uF)-blur(vF)   (TP/TM via +/- band 2-src)
  pass2 (col blur, transposing):  psS, psD, psP, psM
  u = psS^2, v = psD^2 ; a1 = u-v (= 4 mu_x mu_y), b1 = u+v
  Nt = (a1+CA)(psM*g2 - a1 + CB) = 4*A1*A2     (one fused custom-DVE op)
  Dt = (b1+CA)(psP*g2 - b1 + CB) = 4*B1*B2     (same op)
  loss = 1 - mean(Nt/Dt)
g2 corrects the bf16 band's row-sum gain: sigma terms mix once-blurred (gain G)
with squared-blurred (gain G^2) quantities; scaling psP/psM by G re-matches
them (without it the bias is ~4.6e-3 on the loss; with it ~3e-4).

All matmuls bf16 (1 cyc/col vs fp32's 4), banded single-window-per-k-tile
("b4": 542 streamed cols per conv vs 2048 dense).  The divide+multiply+
reduce runs as ONE custom-DVE op (BITWISE_NOT-seeded reciprocal, 1 inline NR
pass, ~1.7e-3 max rel err — noise for a 4M-pixel mean).  Engine split:
PE convs; DVE F1/F2/uF prep, (b1|a1), (Dt|Nt), div+reduce; Act vF, pass-1
drains, (u|v) squares; SP DMA.  Measured ~122 us/core (baseline fp32: 477).
"""

import sys

for _p in ("/opt/trn_rl_repo",):
    if _p not in sys.path:
        sys.path.insert(0, _p)

import numpy as np
import ml_dtypes

import concourse.bass as bass
import concourse.bacc as bacc
import concourse.mybir as mybir
import concourse.tile as tile
from concourse.bass_utils import run_bass_kernel_spmd

F32 = mybir.dt.float32
BF16 = mybir.dt.bfloat16
AOP = mybir.AluOpType
AFT = mybir.ActivationFunctionType

N_CORES = 8
BATCH = 16
CH = 3
H = W = 512
PLANES = (BATCH // N_CORES) * CH  # 6 planes per core
WIN_SIZE = 11
SIGMA = 1.5
HALF = WIN_SIZE // 2
C1 = 0.01 ** 2
C2 = 0.03 ** 2
CA = 2.0 * C1
CB = 2.0 * C2

# banded window per k-tile: output rows [ns, ns+w) reachable from that k-tile
BWIN = [(0, 133), (123, 138), (251, 138), (379, 133)]
BOFF = [0, 133, 271, 409]
CATW = 542

# 1-pass Chebyshev pair for the BITWISE_NOT reciprocal seed
DVA_C0 = -0.23549792
DVA_C1 = 2.0017324


def _gauss1d():
    coords = np.arange(WIN_SIZE, dtype=np.float32) - HALF
    g = np.exp(-(coords ** 2) / np.float32(2.0 * SIGMA ** 2)).astype(np.float32)
    return (g / g.sum(dtype=np.float32)).astype(np.float32)


def _band_full():
    g = _gauss1d()
    A = np.zeros((H, H), dtype=np.float32)
    for i in range(H):
        lo, hi = max(0, i - HALF), min(H, i + HALF + 1)
        for j in range(lo, hi):
            A[i, j] = g[j - i + HALF]
    return A


def band_banded_bf16():
    """[128, 2*542] bf16: banded concat segments then negated copy."""
    A = _band_full()
    segs = []
    for kt in range(4):
        ns, w = BWIN[kt]
        segs.append(A[ns:ns + w, kt * 128:(kt + 1) * 128].T.copy())
    cat = np.concatenate(segs, axis=1)
    assert cat.shape == (128, CATW)
    return np.concatenate([cat, -cat], axis=1).astype(ml_dtypes.bfloat16)


def _gain2():
    """Interior blur2 gain of the bf16 band (row-sum squared)."""
    g = _gauss1d().astype(ml_dtypes.bfloat16).astype(np.float64)
    s1 = float(g.sum())
    return s1 * s1


_NT_OP = None
_DVA_OP = None


def _register_nt_op():
    """out = (in1+s0)*(in0*imm2 - in1 + s1) — the fused SSIM numerator/
    denominator term. Registered at the first free custom-DVE opcode row."""
    global _NT_OP
    if _NT_OP is not None:
        return _NT_OP
    from concourse.dve_ops import DveOp, OPS, _SUB_OPCODE_FOR_NAME, has_src1
    from concourse.dve_spec import Spec, Src0, Src1, C0, C1 as SC1, C2 as SC2, lower
    from concourse.dve_uop import DveOpSpec

    name = "NT_SSIM_ANT"
    if name in _SUB_OPCODE_FOR_NAME:
        _NT_OP = next(o for o in OPS if o.name == name)
        return _NT_OP
    row = max(_SUB_OPCODE_FOR_NAME.values()) + 1
    assert row < 0x20, "no free custom-DVE opcode row"

    def _ref(in0, in1, s0, s1, imm2):
        a = in1.astype(np.float32)
        return (a + s0) * (in0.astype(np.float32) * imm2 - a + s1)

    spec = Spec(body=(Src1 + C0) * (Src0 * SC2 - Src1 + SC1), reference=_ref)
    shas = {}
    for ver in ("v3",):
        s = DveOpSpec(name=name, opcode=row,
                      uops=lower(spec, ver=ver), rd1_en=has_src1(spec))
        shas[ver] = s.sha(ver)
    _SUB_OPCODE_FOR_NAME[name] = row
    op = DveOp(name, spec, subdim=False, uops_sha=shas)
    OPS.append(op)
    _NT_OP = op
    return op


def _register_dva_op():
    """out = Src1 * recip1(Src0); accum_out = sum(out).  recip1 = the
    BITWISE_NOT-seeded approximate reciprocal with one inline NR pass."""
    global _DVA_OP
    if _DVA_OP is not None:
        return _DVA_OP
    from concourse.dve_ops import DveOp, OPS, _SUB_OPCODE_FOR_NAME, has_src1
    from concourse.dve_spec import (
        Spec, Src0, Src1, C0, C1 as SC1, lower, Bin, AluOp,
    )
    from concourse.dve_uop import DveOpSpec
    import operator

    name = "DIV_MUL_ACC_ANT"
    if name in _SUB_OPCODE_FOR_NAME:
        _DVA_OP = next(o for o in OPS if o.name == name)
        return _DVA_OP
    row = max(_SUB_OPCODE_FOR_NAME.values()) + 1
    assert row < 0x20

    _nx = Bin(AluOp.BITWISE_NOT, Src0, Src0)
    _z0 = _nx * C0
    _z1 = _z0 * (SC1 - Src0 * _z0)
    spec = Spec(body=Src1 * _z1, accum=operator.add)
    shas = {}
    for ver in ("v3",):
        s = DveOpSpec(name=name, opcode=row,
                      uops=lower(spec, ver=ver), rd1_en=has_src1(spec))
        shas[ver] = s.sha(ver)
    _SUB_OPCODE_FOR_NAME[name] = row
    op = DveOp(name, spec, subdim=False, uops_sha=shas)
    OPS.append(op)
    _DVA_OP = op
    return op


def build_nc(planes=PLANES):
    nt_op = _register_nt_op()
    dva_op = _register_dva_op()
    g2 = _gain2()

    nc = bacc.Bacc(None)
    dmae = nc.sync
    pred_d = nc.declare_dram_parameter("pred", [planes, H, W], F32, isOutput=False)
    targ_d = nc.declare_dram_parameter("target", [planes, H, W], F32, isOutput=False)
    band_d = nc.declare_dram_parameter("bandmat", [128, 2 * CATW], BF16,
                                       isOutput=False)
    acc_d = nc.declare_dram_parameter("acc", [128, 2 * planes], F32, isOutput=True)

    with tile.TileContext(nc) as tc:
        with (
            tc.tile_pool(name="const", bufs=1) as constp,
            tc.tile_pool(name="xy", bufs=2) as xyp,
            tc.tile_pool(name="fields", bufs=3) as fldp,
            tc.tile_pool(name="tt", bufs=3) as ttp,
            tc.tile_pool(name="post", bufs=2) as pp,
            tc.tile_pool(name="postf", bufs=1) as ppf,
            tc.tile_pool(name="ndp", bufs=2) as ndp,
            tc.tile_pool(name="accp", bufs=1) as accp,
            tc.tile_pool(name="ps1", bufs=2, space="PSUM") as ps1,
            tc.tile_pool(name="ps2", bufs=2, space="PSUM") as ps2,
        ):
            BM = constp.tile([128, 2 * CATW], BF16)
            # Act HWDGE queue: keeps the sync queue free for plane-0 X/Y
            nc.scalar.dma_start(BM[:], band_d[:])
            acc = accp.tile([128, 2 * planes], F32)

            def conv_pass(dst_psum, srcs, blk):
                """dst_psum[128, 512] += transposing 1-D conv along partition
                axis of each (tile, negated) source's 128-col block `blk`."""
                mms = []
                for si, (T, neg) in enumerate(srcs):
                    for kt in range(4):
                        lhsT = T[:, kt * 512 + blk * 128: kt * 512 + (blk + 1) * 128]
                        ns, w = BWIN[kt]
                        off = BOFF[kt] + (CATW if neg else 0)
                        mms.append((dst_psum[:, ns:ns + w],
                                    lhsT, BM[:, off:off + w]))
                n_mm = len(mms)
                for i, (o, l, r) in enumerate(mms):
                    nc.tensor.matmul(o, l, r, start=(i == 0), stop=(i == n_mm - 1))

            for p in range(planes):
                X = xyp.tile([128, 2048], F32, tag="X")
                Y = xyp.tile([128, 2048], F32, tag="Y")
                dmae.dma_start(
                    X[:].rearrange("q (kt c) -> q kt c", kt=4),
                    pred_d[p].rearrange("(kt q) c -> q kt c", q=128))
                dmae.dma_start(
                    Y[:].rearrange("q (kt c) -> q kt c", kt=4),
                    targ_d[p].rearrange("(kt q) c -> q kt c", q=128))

                F1 = fldp.tile([128, 2048], BF16, tag="F1")
                F2 = fldp.tile([128, 2048], BF16, tag="F2")
                uF = fldp.tile([128, 2048], BF16, tag="uF")
                vF = fldp.tile([128, 2048], BF16, tag="vF")
                nc.vector.tensor_tensor(F1[:], X[:], Y[:], AOP.add)
                nc.vector.tensor_tensor(F2[:], X[:], Y[:], AOP.subtract)
                nc.vector.tensor_tensor(uF[:], F1[:], F1[:], AOP.mult)
                nc.scalar.activation(vF[:], F2[:], AFT.Square)

                # pass 1: row blur (transposing), [128,1024] PSUM grain
                p1_specs = [
                    ("Ts", [(F1, False)]),
                    ("Td", [(F2, False)]),
                    ("TP", [(uF, False), (vF, False)]),
                    ("TM", [(uF, False), (vF, True)]),
                ]
                Ts = {}
                for nm, srcs in p1_specs:
                    T = ttp.tile([128, 2048], BF16, tag=nm)
                    for half in range(2):
                        ps = ps1.tile([128, 1024], F32, tag="p1")
                        for i in range(2):
                            mc = 2 * half + i
                            conv_pass(ps[:, i * 512:(i + 1) * 512], srcs, mc)
                        nc.scalar.copy(T[:, half * 1024:(half + 1) * 1024], ps[:])
                    Ts[nm] = T

                # pass 2: col blur + fused post-processing
                uv = pp.tile([128, 4096], BF16, tag="uv")
                ba = pp.tile([128, 4096], BF16, tag="ba")
                ND = ndp.tile([128, 4096], F32, tag="ND")
                scr = ppf.tile([128, 2048], BF16, tag="scr")
                for rc in range(4):
                    o = rc * 1024
                    sl_u = slice(o, o + 512)
                    sl_v = slice(o + 512, o + 1024)
                    psSD = ps2.tile([128, 1024], F32, tag="p2")
                    conv_pass(psSD[:, 0:512], [(Ts["Ts"], False)], rc)
                    conv_pass(psSD[:, 512:1024], [(Ts["Td"], False)], rc)
                    nc.scalar.activation(uv[:, o:o + 1024], psSD[:], AFT.Square)
                    nc.vector.tensor_tensor(
                        ba[:, sl_u], uv[:, sl_u], uv[:, sl_v], AOP.add)
                    nc.vector.tensor_tensor(
                        ba[:, sl_v], uv[:, sl_u], uv[:, sl_v], AOP.subtract)
                    psPM = ps2.tile([128, 1024], F32, tag="p2")
                    conv_pass(psPM[:, 0:512], [(Ts["TP"], False)], rc)
                    conv_pass(psPM[:, 512:1024], [(Ts["TM"], False)], rc)
                    nc.vector._custom_dve(
                        nt_op, out=ND[:, o:o + 1024], in0=psPM[:],
                        in1=ba[:, o:o + 1024], s0=CA, s1=CB, imm2=g2)

                for hf in range(2):
                    hs = slice(hf * 2048, hf * 2048 + 2048)
                    ND_r = ND[:, hs].rearrange("q (r n) -> q r n", r=2)
                    nc.vector._custom_dve(
                        dva_op,
                        out=scr[:, hf * 1024:(hf + 1) * 1024].rearrange(
                            "q (r n) -> q r n", r=2),
                        in0=ND_r[:, :, 0:512], in1=ND_r[:, :, 512:1024],
                        s0=DVA_C0, s1=DVA_C1,
                        accum_out=acc[:, 2 * p + hf:2 * p + hf + 1])

            dmae.dma_start(acc_d[:], acc[:])
    nc.compile()
    return nc


_CACHE = {}


def _get_nc():
    if "nc" not in _CACHE:
        _CACHE["nc"] = build_nc()
        _CACHE["band"] = band_banded_bf16()
    return _CACHE["nc"], _CACHE["band"]


def kernel(pred, target, _trace=False):
    pred = np.ascontiguousarray(np.asarray(pred), dtype=np.float32)
    target = np.ascontiguousarray(np.asarray(target), dtype=np.float32)
    nc, band = _get_nc()
    per = BATCH // N_CORES
    in_maps = []
    for i in range(N_CORES):
        in_maps.append({
            "pred": np.ascontiguousarray(
                pred[per * i: per * (i + 1)].reshape(PLANES, H, W)),
            "target": np.ascontiguousarray(
                target[per * i: per * (i + 1)].reshape(PLANES, H, W)),
            "bandmat": band,
        })
    kw = {}
    if _trace:
        kw["trace"] = True
    res = run_bass_kernel_spmd(nc, in_maps, list(range(N_CORES)), **kw)
    total = 0.0
    for r in res.results:
        total += float(np.asarray(r["acc"]).astype(np.float64).sum())
    loss = 1.0 - total / float(BATCH * CH * H * W)
    out = np.float32(loss)
    if _trace:
        return out, res
    return out
